# revision 3
# baseline (speedup 1.0000x reference)
"""Bitnet-style GQA attention block on 8 trn2 NeuronCores.

Sharding: DP2 (batch) x TP4 (heads). Each core handles one batch element and
8 q-heads / 2 kv-heads. Device layout is feature-major: activations live as
[channels, tokens]; all matmuls are bf16 with fp32 PSUM accumulation.

v2 design (vs the v1 baseline at ~428us):
- The scalar engine's exp stream (256 x [128,1024] ACTIVATEs ~= 285us busy) is
  the hard floor; everything is scheduled to keep it dense from ~20us on.
- Input DMA is column-blocked (xt in 4 token blocks, wq in per-pair chunks)
  so K-proj/Q-proj/first scores start at ~8-15us instead of ~35-70us.
- Scores are row-tiled: per k-chunk, the two kv-heads run as two concurrent
  64-contraction matmuls on PE row halves (tile_position (0,0)/(64,0)),
  halving score PE time vs the zero-padded full-128 scheme.
- PV is operand-swapped: stationary = [V|1] token-major [128,65], moving =
  the exp tile [128,512] per head. Output accumulates attn.T[d,q] (+ a fused
  denominator row) directly in PSUM [65,512] - no per-chunk LDWEIGHTS of P
  tiles (v1 had 2048 of them) and no A-transposes before o-proj.
- PV for window w (a (qb,t) pair) runs one window late (lag-16) so V-proj
  has time to stream in behind the first windows; pa PSUM then needs only
  2 banks. PSUM: scores ping-pong 2x[128,1024] (4) + pa 2x[65,512] (2) +
  scratch 2x[128,512] (2) = 8 banks.
- Normalize: denom row -> gpsimd partition_broadcast -> DVE reciprocal ->
  DVE multiply into the o-proj moving tiles. h=1's rows must land on SBUF
  partitions 64-127, which DVE cannot do (no partition shift), so h=1 goes
  through a small SBUF->SBUF DMA shift.
- o-proj consumes attn.T tiles directly; output is bf16 (host sums partials
  in f32), halving output DMA.
"""

import numpy as np
import ml_dtypes
from contextlib import ExitStack

import concourse.bass as bass
import concourse.tile as tile
from concourse import bacc, mybir
from concourse.bass_utils import run_bass_kernel_spmd
from concourse.masks import make_identity

B, S, H = 2, 2048, 2048
N_HEADS, N_KV, HEAD_DIM = 32, 8, 64
N_CORES = 8
TP = 4                   # head-parallel degree per batch
QH = N_HEADS // TP       # 8 q-heads per core
KVH = N_KV // TP         # 2 kv heads per core
QCH = QH * HEAD_DIM      # 512
KCH = KVH * HEAD_DIM     # 128
ST = S // 128            # 16 k-token chunks
HK = H // 128            # 16 hidden-dim chunks
QB = 4                   # 512-wide q/token column blocks
HEAD_ORDER = [0, 4, 1, 5, 2, 6, 3, 7]  # slot j -> local q-head index

F32 = mybir.dt.float32
BF16 = mybir.dt.bfloat16
BF16_NP = ml_dtypes.bfloat16

_CACHED_NC = None


def _build_nc():
    nc = bacc.Bacc("TRN2", target_bir_lowering=False, debug=False,
                   num_devices=N_CORES)

    xT = nc.dram_tensor("xT", [H, S], BF16, kind="ExternalInput").ap()
    wqT = nc.dram_tensor("wqT", [H, QCH], BF16, kind="ExternalInput").ap()
    wkT = nc.dram_tensor("wkT", [H, KCH], BF16, kind="ExternalInput").ap()
    wvT = nc.dram_tensor("wvT", [H, KCH], BF16, kind="ExternalInput").ap()
    woT = nc.dram_tensor("woT", [QCH, H], BF16, kind="ExternalInput").ap()
    outT = nc.dram_tensor("outT", [H, S], BF16, kind="ExternalOutput").ap()

    with tile.TileContext(nc) as tc, ExitStack() as ctx:
        # ---- SBUF pools ----
        xp = ctx.enter_context(tc.tile_pool(name="xp", bufs=HK * 4))
        wqp = ctx.enter_context(tc.tile_pool(name="wqp", bufs=HK * 4))
        wkp = ctx.enter_context(tc.tile_pool(name="wkp", bufs=HK))
        wvp = ctx.enter_context(tc.tile_pool(name="wvp", bufs=HK))
        wop = ctx.enter_context(tc.tile_pool(name="wop", bufs=4))
        ktp = ctx.enter_context(tc.tile_pool(name="ktp", bufs=1))
        qtp = ctx.enter_context(tc.tile_pool(name="qtp", bufs=4))
        vp = ctx.enter_context(tc.tile_pool(name="vp", bufs=ST))
        pexp = ctx.enter_context(tc.tile_pool(name="pexp", bufs=18))
        atp = ctx.enter_context(tc.tile_pool(name="atp", bufs=8))
        stg = ctx.enter_context(tc.tile_pool(name="stg", bufs=4))
        nrm = ctx.enter_context(tc.tile_pool(name="nrm", bufs=2))
        cst = ctx.enter_context(tc.tile_pool(name="cst", bufs=1))
        # PSUM: scores ping-pong (4 banks) + pa (2) + scratch (2) = 8
        big = ctx.enter_context(tc.tile_pool(name="big", bufs=2, space="PSUM"))
        pap = ctx.enter_context(tc.tile_pool(name="pap", bufs=2, space="PSUM"))
        acc = ctx.enter_context(tc.tile_pool(name="acc", bufs=2, space="PSUM"))

        ident = cst.tile([128, 128], BF16, tag="ident")
        make_identity(nc, ident[:])

        # prime the ACT exp table while input DMA streams
        dum = cst.tile([1, 2], F32, tag="dum")
        dumo = cst.tile([1, 2], BF16, tag="dumo")
        nc.vector.memset(dum[:], 0.0)
        nc.scalar.activation(dumo[:], dum[:],
                             mybir.ActivationFunctionType.Exp)

        # ---- input DMA: column-blocked, on sync+gpsimd rings ----
        rings = [nc.sync, nc.gpsimd]

        wk, wv = [], []
        xts = [[None] * HK for _ in range(4)]        # xts[sb][hk]
        wq = [[None] * 4 for _ in range(HK)]         # wq[hk][t]
        wo = []

        def dma_wk():
            for i in range(HK):
                t = wkp.tile([128, KCH], BF16, tag="wk", name=f"wk{i}")
                rings[i % 2].dma_start(t[:], wkT[i * 128:(i + 1) * 128, :])
                wk.append(t)

        def dma_wv():
            for i in range(HK):
                t = wvp.tile([128, KCH], BF16, tag="wv", name=f"wv{i}")
                rings[i % 2].dma_start(t[:], wvT[i * 128:(i + 1) * 128, :])
                wv.append(t)

        def dma_xt(sb):
            for hk in range(HK):
                t = xp.tile([128, 512], BF16, tag="xt", name=f"x{sb}_{hk}")
                rings[hk % 2].dma_start(
                    t[:], xT[hk * 128:(hk + 1) * 128, sb * 512:(sb + 1) * 512])
                xts[sb][hk] = t

        def dma_wq(t_):
            for hk in range(HK):
                w = wqp.tile([128, 128], BF16, tag="wq", name=f"wq{hk}_{t_}")
                rings[hk % 2].dma_start(
                    w[:], wqT[hk * 128:(hk + 1) * 128, t_ * 128:(t_ + 1) * 128])
                wq[hk][t_] = w

        def dma_wo():
            for i in range(4):
                t = wop.tile([128, H], BF16, tag="wo", name=f"wo{i}")
                rings[i % 2].dma_start(t[:], woT[i * 128:(i + 1) * 128, :])
                wo.append(t)

        # priority order: first compute needs wk, xt0, wq_t0
        dma_wk()
        dma_xt(0)
        dma_wq(0)
        dma_wq(1)
        dma_wv()
        dma_xt(1)
        dma_wq(2)
        dma_xt(2)
        dma_xt(3)
        dma_wq(3)
        dma_wo()

        # ---- persistent SBUF tensors ----
        kt_sb = ktp.tile([128, S], BF16, tag="kt")
        qt = [qtp.tile([128, S], BF16, tag="qt", name=f"qt{t_}")
              for t_ in range(4)]
        vones = [vp.tile([128, 130], BF16, tag="vones", name=f"vt{st}")
                 for st in range(ST)]
        for st in range(ST):
            nc.gpsimd.memset(vones[st][:, 64:65], 1.0)
            nc.gpsimd.memset(vones[st][:, 129:130], 1.0)

        # ---- projection sub-blocks (emitted via the injection schedule) ----
        pk_h, pq_h, pvt_h, vtsb_h = {}, {}, {}, {}

        def emit_kproj(sb, half):
            if half == 0:
                pk_h[sb] = acc.tile([128, 512], F32, tag="acc", name=f"pk{sb}")
            pk = pk_h[sb]
            for hk in range(half * 8, half * 8 + 8):
                nc.tensor.matmul(pk[:], wk[hk][:], xts[sb][hk][:],
                                 start=(hk == 0), stop=(hk == HK - 1),
                                 skip_group_check=True)
            if half == 1:
                nc.vector.tensor_copy(kt_sb[:, sb * 512:(sb + 1) * 512], pk[:])

        def emit_qproj(t_, sb, half):
            if half == 0:
                pq_h[(t_, sb)] = acc.tile([128, 512], F32, tag="acc",
                                          name=f"pq{t_}_{sb}")
            pq = pq_h[(t_, sb)]
            for hk in range(half * 8, half * 8 + 8):
                nc.tensor.matmul(pq[:], wq[hk][t_][:], xts[sb][hk][:],
                                 start=(hk == 0), stop=(hk == HK - 1),
                                 skip_group_check=True)
            if half == 1:
                nc.vector.tensor_copy(qt[t_][:, sb * 512:(sb + 1) * 512], pq[:])

        def emit_vproj(sb, part):
            if part == 0:
                pvt_h[sb] = acc.tile([128, 512], F32, tag="acc", name=f"pv{sb}")
            if part in (0, 1):
                pvt = pvt_h[sb]
                for hk in range(part * 8, part * 8 + 8):
                    nc.tensor.matmul(pvt[:], wv[hk][:], xts[sb][hk][:],
                                     start=(hk == 0), stop=(hk == HK - 1),
                                     skip_group_check=True)
            if part == 1:
                vtsb_h[sb] = stg.tile([128, 512], BF16, tag="vtsb",
                                      name=f"vtsb{sb}")
                nc.vector.tensor_copy(vtsb_h[sb][:], pvt_h[sb][:])
            if part == 2:
                vtsb = vtsb_h[sb]
                for j in range(4):
                    st = sb * 4 + j
                    ptr = acc.tile([128, 128], BF16, tag="acc", name="ptv")
                    nc.tensor.transpose(ptr[:], vtsb[:, j * 128:(j + 1) * 128],
                                        ident[:])
                    nc.vector.tensor_copy(vones[st][:, 0:64], ptr[:, 0:64])
                    nc.vector.tensor_copy(vones[st][:, 65:129], ptr[:, 64:128])

        at_of = {}

        def emit_oproj_ot(qb, ot):
            po = acc.tile([128, 512], F32, tag="acc", name="po")
            for ak in range(4):
                nc.tensor.matmul(po[:], wo[ak][:, ot * 128:(ot + 1) * 128],
                                 at_of[qb][ak][:],
                                 start=(ak == 0), stop=(ak == 3),
                                 skip_group_check=True)
            so = stg.tile([128, 512], BF16, tag="so")
            nc.vector.tensor_copy(so[:], po[:])
            nc.sync.dma_start(
                outT[ot * 128:(ot + 1) * 128, qb * 512:(qb + 1) * 512], so[:])

        # ---- attention plumbing ----
        def emit_pv(ptiles, pa, kt):
            # attn.T[d,q] (+denominator row 64) accumulated over k chunks:
            # stationary [V|1] token-major, moving = exp tile half.
            for h in range(2):
                nc.tensor.matmul(pa[h][:, :],
                                 vones[kt][:, h * 65:h * 65 + 65],
                                 ptiles[kt][:, h * 512:(h + 1) * 512],
                                 start=(kt == 0), stop=(kt == ST - 1),
                                 skip_group_check=True)

        def emit_normalize(pqb, pt_, pa):
            # at[qb][t] rows h*64:(h+1)*64 = pa[h] rows 0:64 / denom row 64.
            for h in range(2):
                dcp = nrm.tile([1, 512], F32, tag="dcp")
                nc.vector.tensor_copy(dcp[:], pa[h][64:65, :])
                dbc = nrm.tile([64, 512], F32, tag="dbc")
                nc.gpsimd.partition_broadcast(dbc[:], dcp[:])
                rct = nrm.tile([64, 512], F32, tag="rct")
                nc.vector.reciprocal(rct[:], dbc[:])
                if h == 0:
                    nc.vector.tensor_mul(at_of[pqb][pt_][0:64, :],
                                         pa[h][0:64, :], rct[:])
                else:
                    # DVE cannot shift partitions; bounce via SBUF DMA
                    tmp = nrm.tile([64, 512], BF16, tag="tmp")
                    nc.vector.tensor_mul(tmp[:], pa[h][0:64, :], rct[:])
                    nc.sync.dma_start(at_of[pqb][pt_][64:128, :], tmp[:])

        # ---- injection schedule: (qb, t) -> sub-block thunks, one per chunk
        K = emit_kproj
        Q = emit_qproj
        V = emit_vproj
        O = emit_oproj_ot

        def KB(sb, h):
            return lambda: K(sb, h)

        def QB_(t_, sb, h):
            return lambda: Q(t_, sb, h)

        def VB(sb, p):
            return lambda: V(sb, p)

        def OB(qb, ot):
            return lambda: O(qb, ot)

        sched = {
            (0, 0): [KB(1, 0), KB(1, 1), VB(1, 0), VB(1, 1), VB(1, 2),
                     KB(2, 0), KB(2, 1), QB_(1, 0, 0), QB_(1, 0, 1),
                     KB(3, 0), KB(3, 1)],
            (0, 1): [VB(2, 0), VB(2, 1), VB(2, 2), QB_(2, 0, 0), QB_(2, 0, 1),
                     VB(3, 0), VB(3, 1), VB(3, 2)],
            (0, 2): [QB_(3, 0, 0), QB_(3, 0, 1)],
            (0, 3): [QB_(0, 1, 0), QB_(0, 1, 1)],
            (1, 0): [QB_(1, 1, 0), QB_(1, 1, 1)],
            (1, 1): [QB_(2, 1, 0), QB_(2, 1, 1)] + [OB(0, i) for i in range(5)],
            (1, 2): [QB_(3, 1, 0), QB_(3, 1, 1)] + [OB(0, i) for i in range(5, 10)],
            (1, 3): [QB_(0, 2, 0), QB_(0, 2, 1)] + [OB(0, i) for i in range(10, 16)],
            (2, 0): [QB_(1, 2, 0), QB_(1, 2, 1)],
            (2, 1): [QB_(2, 2, 0), QB_(2, 2, 1)] + [OB(1, i) for i in range(5)],
            (2, 2): [QB_(3, 2, 0), QB_(3, 2, 1)] + [OB(1, i) for i in range(5, 10)],
            (2, 3): [QB_(0, 3, 0), QB_(0, 3, 1)] + [OB(1, i) for i in range(10, 16)],
            (3, 0): [QB_(1, 3, 0), QB_(1, 3, 1)],
            (3, 1): [QB_(2, 3, 0), QB_(2, 3, 1)] + [OB(2, i) for i in range(5)],
            (3, 2): [QB_(3, 3, 0), QB_(3, 3, 1)] + [OB(2, i) for i in range(5, 10)],
            (3, 3): [OB(2, i) for i in range(10, 16)],
        }

        # ---- pre-loop: K sb0, Q(0,0), V sb0 ----
        emit_kproj(0, 0)
        emit_kproj(0, 1)
        emit_qproj(0, 0, 0)
        emit_qproj(0, 0, 1)
        emit_vproj(0, 0)
        emit_vproj(0, 1)
        emit_vproj(0, 2)

        # ---- main loop: qb outer, head-pair t inner; PV runs one window late
        prev = None  # (ptiles, qb, t)
        for qb in range(QB):
            at_of[qb] = [atp.tile([128, 512], BF16, tag="at",
                                  name=f"at{qb}_{ak}") for ak in range(4)]
            for t_ in range(4):
                inj = list(sched.get((qb, t_), []))
                cur = []
                if prev is not None:
                    pa = [pap.tile([65, 512], F32, tag="pa", name=f"pa{h}")
                          for h in range(2)]
                for kt in range(ST):
                    ps2 = big.tile([128, 1024], F32, tag="big")
                    for h in range(2):
                        nc.tensor.matmul(
                            ps2[:, h * 512:(h + 1) * 512],
                            kt_sb[h * 64:(h + 1) * 64,
                                  kt * 128:(kt + 1) * 128],
                            qt[t_][h * 64:(h + 1) * 64,
                                   qb * 512:(qb + 1) * 512],
                            start=True, stop=True)
                    pe = pexp.tile([128, 1024], BF16, tag="pexp")
                    nc.scalar.activation(pe[:], ps2[:],
                                         mybir.ActivationFunctionType.Exp,
                                         scale=0.125)
                    cur.append(pe)
                    if prev is not None:
                        emit_pv(prev[0], pa, kt)
                    if inj:
                        inj.pop(0)()
                for f in inj:
                    f()
                if prev is not None:
                    emit_normalize(prev[1], prev[2], pa)
                prev = (cur, qb, t_)

        # ---- tail: PV + normalize of the last window, then o-proj qb3 ----
        pa = [pap.tile([65, 512], F32, tag="pa", name=f"paz{h}")
              for h in range(2)]
        for kt in range(ST):
            emit_pv(prev[0], pa, kt)
        emit_normalize(prev[1], prev[2], pa)
        for ot in range(HK):
            emit_oproj_ot(QB - 1, ot)

    nc.compile()
    return nc


def _get_nc():
    global _CACHED_NC
    if _CACHED_NC is None:
        _CACHED_NC = _build_nc()
    return _CACHED_NC


def _prep_core_inputs(hidden_states, Wq, Wk, Wv, Wo):
    """Host-side shard + transpose + bf16 cast. Returns list of 8 input dicts."""
    xT_b = []
    for b in range(B):
        xT_b.append(np.ascontiguousarray(hidden_states[b].T).astype(BF16_NP))
    in_maps = []
    for c in range(N_CORES):
        b, g = divmod(c, TP)
        wq_rows = np.concatenate([
            Wq[(g * QH + h) * HEAD_DIM:(g * QH + h + 1) * HEAD_DIM, :]
            for h in HEAD_ORDER], axis=0)            # [512, H]
        wo_cols = np.concatenate([
            Wo[:, (g * QH + h) * HEAD_DIM:(g * QH + h + 1) * HEAD_DIM]
            for h in HEAD_ORDER], axis=1)            # [H, 512]
        in_maps.append({
            "xT": xT_b[b],
            "wqT": np.ascontiguousarray(wq_rows.T).astype(BF16_NP),
            "wkT": np.ascontiguousarray(Wk[g * KCH:(g + 1) * KCH, :].T).astype(BF16_NP),
            "wvT": np.ascontiguousarray(Wv[g * KCH:(g + 1) * KCH, :].T).astype(BF16_NP),
            "woT": np.ascontiguousarray(wo_cols.T).astype(BF16_NP),
        })
    return in_maps


def _combine(results):
    out = np.empty((B, S, H), dtype=np.float32)
    for b in range(B):
        acc = results[b * TP]["outT"].astype(np.float32)
        for g in range(1, TP):
            acc = acc + results[b * TP + g]["outT"]
        out[b] = acc.T
    return out


def kernel(hidden_states, attention_mask, Wq, Wk, Wv, Wo):
    # attention_mask is all zeros for this problem spec; softmax is invariant
    # to the zero additive mask, so it is not shipped to the device.
    hidden_states = np.asarray(hidden_states)
    nc = _get_nc()
    in_maps = _prep_core_inputs(hidden_states, np.asarray(Wq), np.asarray(Wk),
                                np.asarray(Wv), np.asarray(Wo))
    res = run_bass_kernel_spmd(nc, in_maps, list(range(N_CORES)))
    return _combine(res.results)


# revision 23
# speedup vs baseline: 1.1793x; 1.1793x over previous
"""Bitnet-style GQA attention block on 8 trn2 NeuronCores.

Sharding: DP2 (batch) x TP4 (heads). Each core handles one batch element and
8 q-heads / 2 kv-heads. Device layout is feature-major: activations live as
[channels, tokens]; all matmuls are bf16 with fp32 PSUM accumulation.

v3 design (vs the v1 baseline at ~428us):
- The scalar engine's exp stream (256 x [128,1024] ACTIVATEs ~= 285us busy) is
  the hard floor; everything is scheduled to keep it dense from ~16us on.
- Input DMA uses one 3D descriptor per block (multi-hk rearranged APs) so the
  queue-kick latency is negligible, ordered so K/Q-proj and the first scores
  start ~8-16us in. The vones ones-column memsets are emitted before any
  gpsimd-ring DMA so the first PV is not stuck behind descriptor kicks.
- Scores are row-tiled: per k-chunk, the two kv-heads run as two concurrent
  64-contraction matmuls on PE row halves (tile_position (0,0)/(64,0)),
  halving score PE time vs the zero-padded full-128 scheme.
- PV is operand-swapped: stationary = [V|1] token-major [128,65], moving =
  the exp tile [128,512] per head. Output accumulates attn.T[d,q] (+ a fused
  denominator row) directly in PSUM [65,512] - no per-chunk LDWEIGHTS of P
  tiles (v1 had 2048 of them) and no A-transposes before o-proj.
- PV for window w (a (qb,t) pair) runs one window late (lag-16) so V-proj
  can stream in behind the first windows; pa PSUM then needs only 2 banks.
  PSUM: scores ping-pong 2x[128,1024] (4) + pa 2x[65,512] (2) + scratch
  2x[128,512] (2) = 8 banks.
- Normalize: pa is freed by two quick DVE copies (attn rows + denom row);
  the rest (gpsimd partition_broadcast -> reciprocal_approx_fast -> multiply
  into the o-proj moving tiles) runs off the critical path. h=1's rows must
  land on SBUF partitions 64-127, which DVE cannot write (no partition
  shift), so h=1 bounces through a small SBUF->SBUF DMA.
- o-proj consumes attn.T tiles directly; output is bf16 (host sums partials
  in f32), halving output DMA.
"""

import numpy as np
import ml_dtypes
from contextlib import ExitStack

import concourse.bass as bass
import concourse.tile as tile
from concourse import bacc, mybir
from concourse.bass_utils import run_bass_kernel_spmd
from concourse.masks import make_identity

B, S, H = 2, 2048, 2048
N_HEADS, N_KV, HEAD_DIM = 32, 8, 64
N_CORES = 8
TP = 4                   # head-parallel degree per batch
QH = N_HEADS // TP       # 8 q-heads per core
KVH = N_KV // TP         # 2 kv heads per core
QCH = QH * HEAD_DIM      # 512
KCH = KVH * HEAD_DIM     # 128
ST = S // 128            # 16 k-token chunks
HK = H // 128            # 16 hidden-dim chunks
QB = 4                   # 512-wide q/token column blocks
HEAD_ORDER = [0, 4, 1, 5, 2, 6, 3, 7]  # slot j -> local q-head index

F32 = mybir.dt.float32
BF16 = mybir.dt.bfloat16
BF16_NP = ml_dtypes.bfloat16

_CACHED_NC = None


def _build_nc():
    nc = bacc.Bacc("TRN2", target_bir_lowering=False, debug=False,
                   num_devices=N_CORES)

    xT = nc.dram_tensor("xT", [H, S], BF16, kind="ExternalInput").ap()
    wqT = nc.dram_tensor("wqT", [H, QCH], BF16, kind="ExternalInput").ap()
    wkT = nc.dram_tensor("wkT", [H, KCH], BF16, kind="ExternalInput").ap()
    wvT = nc.dram_tensor("wvT", [H, KCH], BF16, kind="ExternalInput").ap()
    woT = nc.dram_tensor("woT", [QCH, H], BF16, kind="ExternalInput").ap()
    outT = nc.dram_tensor("outT", [H, S], BF16, kind="ExternalOutput").ap()

    # multi-hk 3D views: one DMA descriptor per block
    x3 = xT.rearrange("(hk p) s -> p hk s", p=128)
    wq3 = wqT.rearrange("(hk p) c -> p hk c", p=128)
    wk3 = wkT.rearrange("(hk p) c -> p hk c", p=128)
    wv3 = wvT.rearrange("(hk p) c -> p hk c", p=128)

    with tile.TileContext(nc) as tc, ExitStack() as ctx:
        # ---- SBUF pools ----
        xp = ctx.enter_context(tc.tile_pool(name="xp", bufs=4))
        wqp = ctx.enter_context(tc.tile_pool(name="wqp", bufs=4))
        wkp = ctx.enter_context(tc.tile_pool(name="wkp", bufs=1))
        wvp = ctx.enter_context(tc.tile_pool(name="wvp", bufs=1))
        wop = ctx.enter_context(tc.tile_pool(name="wop", bufs=4))
        ktp = ctx.enter_context(tc.tile_pool(name="ktp", bufs=1))
        qtp = ctx.enter_context(tc.tile_pool(name="qtp", bufs=4))
        vp = ctx.enter_context(tc.tile_pool(name="vp", bufs=ST))
        pexp = ctx.enter_context(tc.tile_pool(name="pexp", bufs=18))
        atp = ctx.enter_context(tc.tile_pool(name="atp", bufs=8))
        stg = ctx.enter_context(tc.tile_pool(name="stg", bufs=4))
        nrm = ctx.enter_context(tc.tile_pool(name="nrm", bufs=2))
        cst = ctx.enter_context(tc.tile_pool(name="cst", bufs=1))
        # PSUM: scores ping-pong (4 banks) + pa (2) + scratch (2) = 8
        big = ctx.enter_context(tc.tile_pool(name="big", bufs=2, space="PSUM"))
        pap = ctx.enter_context(tc.tile_pool(name="pap", bufs=2, space="PSUM"))
        acc = ctx.enter_context(tc.tile_pool(name="acc", bufs=2, space="PSUM"))

        ident = cst.tile([128, 128], BF16, tag="ident")
        make_identity(nc, ident[:])
        ones64 = cst.tile([1, 64], BF16, tag="ones64")
        nc.vector.memset(ones64[:], 1.0)

        # prime the ACT exp table while input DMA streams
        dum = cst.tile([1, 2], F32, tag="dum")
        dumo = cst.tile([1, 2], BF16, tag="dumo")
        nc.vector.memset(dum[:], 1.0)
        nc.scalar.activation(dumo[:], dum[:],
                             mybir.ActivationFunctionType.Exp)

        # vones tiles; the ones columns are (re)written by DVE memsets AFTER
        # the V copies in emit_vproj - the PV stationary read's hazard
        # interval (reversed weights AP) reliably covers the ones columns,
        # and DVE is in-order, so that memset transitively orders the copies
        # before the PV LDWEIGHTS. (Without it, the hazard tracker misses
        # the copy ranges and the PV can read stale vones - seen on HW.)
        vones = [vp.tile([128, 130], BF16, tag="vones", name=f"vt{st}")
                 for st in range(ST)]

        # ---- input DMA: one 3D descriptor per block ----
        xts = []          # xts[sb]: [128, HK, 512]
        wqt = []          # wqt[t]: [128, HK, 128]
        wo = []

        wk_t = wkp.tile([128, HK, KCH], BF16, tag="wk")
        wv_t = wvp.tile([128, HK, KCH], BF16, tag="wv")
        for sb in range(4):
            t = xp.tile([128, HK, 512], BF16, tag="xt", name=f"xts{sb}")
            xts.append(t)
        for t_ in range(4):
            w = wqp.tile([128, HK, 128], BF16, tag="wq", name=f"wqt{t_}")
            wqt.append(w)

        # sync ring: critical path. wq0 kicks before xt0 so ring order makes
        # the (tracked) xt0 moving-operand dep imply wq0 residency.
        nc.sync.dma_start(wk_t[:, :, :], wk3)
        nc.sync.dma_start(wqt[0][:, :, :], wq3[:, :, 0:128])
        nc.sync.dma_start(xts[0][:, :, :], x3[:, :, 0:512])
        nc.sync.dma_start(xts[1][:, :, :], x3[:, :, 512:1024])
        nc.sync.dma_start(xts[2][:, :, :], x3[:, :, 1024:1536])
        nc.sync.dma_start(xts[3][:, :, :], x3[:, :, 1536:2048])
        # gpsimd ring: lower priority (shares HBM wire)
        nc.gpsimd.dma_start(wv_t[:, :, :], wv3)
        nc.gpsimd.dma_start(wqt[1][:, :, :], wq3[:, :, 128:256])
        nc.gpsimd.dma_start(wqt[2][:, :, :], wq3[:, :, 256:384])
        nc.gpsimd.dma_start(wqt[3][:, :, :], wq3[:, :, 384:512])
        for i in range(4):
            t = wop.tile([128, H], BF16, tag="wo", name=f"wo{i}")
            nc.gpsimd.dma_start(t[:], woT[i * 128:(i + 1) * 128, :])
            wo.append(t)

        # ---- persistent SBUF tensors ----
        kt_sb = ktp.tile([128, S], BF16, tag="kt")
        qt = [qtp.tile([128, S], BF16, tag="qt", name=f"qt{t_}")
              for t_ in range(4)]

        # ---- projection sub-blocks (emitted via the injection schedule) ----
        pk_h, pq_h, pvt_h, vtsb_h = {}, {}, {}, {}

        def guard(src_ap):
            # Tile elides LDWEIGHTS waits when an earlier PE-queue wait
            # covers the same semaphore count - unsound because the PE
            # hoists LDWEIGHTS past in-flight MATMULs (seen on HW: PV read
            # stale vones). This guard loads the freshly written bytes as
            # its *stationary* operand: the fresh dep can't be covered by
            # any earlier wait, so the guard's LDWEIGHTS carries it, and
            # later LDWEIGHTS can't hoist past another LDWEIGHTS.
            g = acc.tile([2, 2], F32, tag="acc", name="guard")
            nc.tensor.matmul(g[:], src_ap, ident[:, 0:2], start=True,
                             stop=True)

        def emit_kproj(sb, half):
            if half == 0:
                pk_h[sb] = acc.tile([128, 512], F32, tag="acc", name=f"pk{sb}")
            pk = pk_h[sb]
            for hk in range(half * 8, half * 8 + 8):
                nc.tensor.matmul(pk[:], wk_t[:, hk, :], xts[sb][:, hk, :],
                                 start=(hk == 0), stop=(hk == HK - 1),
                                 skip_group_check=True)
            if half == 1:
                nc.vector.tensor_copy(kt_sb[:, sb * 512:(sb + 1) * 512], pk[:])
                guard(kt_sb[:, sb * 512 + 510:sb * 512 + 512])

        def emit_qproj(t_, sb, half):
            if half == 0:
                pq_h[(t_, sb)] = acc.tile([128, 512], F32, tag="acc",
                                          name=f"pq{t_}_{sb}")
            pq = pq_h[(t_, sb)]
            for hk in range(half * 8, half * 8 + 8):
                nc.tensor.matmul(pq[:], wqt[t_][:, hk, :], xts[sb][:, hk, :],
                                 start=(hk == 0), stop=(hk == HK - 1),
                                 skip_group_check=True)
            if half == 1:
                nc.vector.tensor_copy(qt[t_][:, sb * 512:(sb + 1) * 512], pq[:])

        def emit_vproj(sb, part):
            if part == 0:
                pvt_h[sb] = acc.tile([128, 512], F32, tag="acc", name=f"pv{sb}")
            if part in (0, 1):
                pvt = pvt_h[sb]
                for hk in range(part * 8, part * 8 + 8):
                    nc.tensor.matmul(pvt[:], wv_t[:, hk, :], xts[sb][:, hk, :],
                                     start=(hk == 0), stop=(hk == HK - 1),
                                     skip_group_check=True)
            if part == 1:
                vtsb_h[sb] = stg.tile([128, 512], BF16, tag="vtsb",
                                      name=f"vtsb{sb}")
                nc.vector.tensor_copy(vtsb_h[sb][:], pvt_h[sb][:])
                guard(vtsb_h[sb][:, 510:512])
            if part == 2:
                vtsb = vtsb_h[sb]
                for j in range(4):
                    st = sb * 4 + j
                    ptr = acc.tile([128, 128], BF16, tag="acc", name="ptv")
                    nc.tensor.transpose(ptr[:], vtsb[:, j * 128:(j + 1) * 128],
                                        ident[:])
                    nc.vector.tensor_copy(vones[st][:, 0:64], ptr[:, 0:64])
                    nc.vector.tensor_copy(vones[st][:, 65:129], ptr[:, 64:128])
                    nc.vector.memset(vones[st][:, 64:65], 1.0)
                    nc.vector.memset(vones[st][:, 129:130], 1.0)
                guard(vones[sb * 4 + 3][:, 128:130])

        at_of = {}

        def emit_oproj_ot(qb, ot):
            po = acc.tile([128, 512], F32, tag="acc", name="po")
            for ak in range(4):
                nc.tensor.matmul(po[:], wo[ak][:, ot * 128:(ot + 1) * 128],
                                 at_of[qb][ak][:],
                                 start=(ak == 0), stop=(ak == 3),
                                 skip_group_check=True)
            so = stg.tile([128, 512], BF16, tag="so")
            nc.vector.tensor_copy(so[:], po[:])
            nc.sync.dma_start(
                outT[ot * 128:(ot + 1) * 128, qb * 512:(qb + 1) * 512], so[:])

        # ---- attention plumbing ----
        def emit_pv(ptiles, pa, kt):
            # attn.T[d,q] (+denominator row 64) accumulated over k chunks:
            # stationary [V|1] token-major, moving = exp tile half.
            for h in range(2):
                nc.tensor.matmul(pa[h][:, :],
                                 vones[kt][:, h * 65:h * 65 + 65],
                                 ptiles[kt][:, h * 512:(h + 1) * 512],
                                 start=(kt == 0), stop=(kt == ST - 1),
                                 skip_group_check=True)

        def emit_normalize(pqb, pt_, pa):
            # Free pa quickly with two copies; the recip+multiply runs off
            # the critical path. at[qb][t] rows h*64.. get pa[h] rows 0:64.
            for h in range(2):
                anT = nrm.tile([64, 512], F32, tag="anT")
                nc.vector.tensor_copy(anT[:], pa[h][0:64, :])
                dcp = nrm.tile([1, 512], BF16, tag="dcp")
                nc.vector.tensor_copy(dcp[:], pa[h][64:65, :])
                # replicate the denominator row across 64 partitions via a
                # K=1 matmul (ones outer product) - cheap, fully tracked
                pbc = acc.tile([64, 512], F32, tag="acc", name="pbc")
                nc.tensor.matmul(pbc[:], ones64[:], dcp[:],
                                 start=True, stop=True)
                rct = nrm.tile([64, 512], F32, tag="rct")
                nc.vector.reciprocal(rct[:], pbc[:])
                if h == 0:
                    nc.vector.tensor_mul(at_of[pqb][pt_][0:64, :],
                                         anT[:], rct[:])
                else:
                    # DVE cannot shift partitions; bounce via SBUF DMA
                    tmp = nrm.tile([64, 512], BF16, tag="tmp")
                    nc.vector.tensor_mul(tmp[:], anT[:], rct[:])
                    nc.sync.dma_start(at_of[pqb][pt_][64:128, :], tmp[:])

        # ---- injection schedule: (qb,t) -> {chunk: [thunks]} ----
        def KB(sb, h):
            return lambda: emit_kproj(sb, h)

        def QP(t_, sb, h):
            return lambda: emit_qproj(t_, sb, h)

        def VB(sb, p):
            return lambda: emit_vproj(sb, p)

        def OB(qb, ot):
            return lambda: emit_oproj_ot(qb, ot)

        def qw(t_, sb):     # standard Q-proj pair placement
            return {2: [QP(t_, sb, 0)], 4: [QP(t_, sb, 1)]}

        def ow(qb, lo, hi, base=6):  # o-proj row-tiles spread over chunks
            out = {}
            for i, ot in enumerate(range(lo, hi)):
                out.setdefault(min(base + 2 * i, ST - 1), []).append(OB(qb, ot))
            return out

        def merge(*ds):
            out = {}
            for dd in ds:
                for k, v in dd.items():
                    out.setdefault(k, []).extend(v)
            return out

        sched = {
            (0, 0): {1: [KB(1, 0)], 2: [KB(1, 1)], 5: [KB(2, 0)],
                     6: [KB(2, 1)], 9: [KB(3, 0)], 10: [KB(3, 1)],
                     12: [QP(1, 0, 0)], 13: [QP(1, 0, 1)],
                     14: [VB(0, 0)], 15: [VB(0, 1)]},
            # PV(prev, kt) is emitted at chunk kt BEFORE the chunk's thunks:
            # VB(sb, 2) (writes vones[4sb..4sb+3]) must sit at chunk < 4*sb.
            (0, 1): {0: [VB(1, 0)], 1: [VB(1, 1)], 2: [VB(1, 2)],
                     3: [QP(2, 0, 0)], 4: [VB(2, 0)], 5: [VB(2, 1)],
                     6: [VB(2, 2)], 7: [QP(2, 0, 1)],
                     8: [VB(3, 0)], 9: [VB(3, 1)], 10: [VB(3, 2)]},
            (0, 2): qw(3, 0),
            (0, 3): qw(0, 1),
            (1, 0): qw(1, 1),
            (1, 1): merge(qw(2, 1), ow(0, 0, 5)),
            (1, 2): merge(qw(3, 1), ow(0, 5, 10)),
            (1, 3): merge(qw(0, 2), ow(0, 10, 16), ),
            (2, 0): qw(1, 2),
            (2, 1): merge(qw(2, 2), ow(1, 0, 5)),
            (2, 2): merge(qw(3, 2), ow(1, 5, 10)),
            (2, 3): merge(qw(0, 3), ow(1, 10, 16)),
            (3, 0): qw(1, 3),
            (3, 1): merge(qw(2, 3), ow(2, 0, 5)),
            (3, 2): merge(qw(3, 3), ow(2, 5, 10)),
            (3, 3): ow(2, 10, 16, base=4),
        }
        # (0,0) tail: V0 part 2 runs right after the chunk loop
        late_of = {(0, 0): [VB(0, 2)]}

        # ---- pre-loop: K sb0 and Q(0,0) ----
        emit_kproj(0, 0)
        emit_kproj(0, 1)
        emit_qproj(0, 0, 0)
        emit_qproj(0, 0, 1)

        # ---- main loop: qb outer, head-pair t inner; PV runs one window late
        prev = None  # (ptiles, qb, t)
        for qb in range(QB):
            at_of[qb] = [atp.tile([128, 512], BF16, tag="at",
                                  name=f"at{qb}_{ak}") for ak in range(4)]
            for t_ in range(4):
                cmap = sched.get((qb, t_), {})
                cur = []
                if prev is not None:
                    pa = [pap.tile([65, 512], F32, tag="pa", name=f"pa{h}")
                          for h in range(2)]
                for kt in range(ST):
                    ps2 = big.tile([128, 1024], F32, tag="big")
                    for h in range(2):
                        nc.tensor.matmul(
                            ps2[:, h * 512:(h + 1) * 512],
                            kt_sb[h * 64:(h + 1) * 64,
                                  kt * 128:(kt + 1) * 128],
                            qt[t_][h * 64:(h + 1) * 64,
                                   qb * 512:(qb + 1) * 512],
                            start=True, stop=True)
                    pe = pexp.tile([128, 1024], BF16, tag="pexp")
                    nc.scalar.activation(pe[:], ps2[:],
                                         mybir.ActivationFunctionType.Exp,
                                         scale=0.125)
                    cur.append(pe)
                    if prev is not None:
                        emit_pv(prev[0], pa, kt)
                    for f in cmap.get(kt, []):
                        f()
                for f in late_of.get((qb, t_), []):
                    f()
                if prev is not None:
                    emit_normalize(prev[1], prev[2], pa)
                prev = (cur, qb, t_)

        # ---- tail: PV + normalize of the last window, then o-proj qb3 ----
        pa = [pap.tile([65, 512], F32, tag="pa", name=f"paz{h}")
              for h in range(2)]
        for kt in range(ST):
            emit_pv(prev[0], pa, kt)
        emit_normalize(prev[1], prev[2], pa)
        for ot in range(HK):
            emit_oproj_ot(QB - 1, ot)

    nc.compile()
    return nc


def _get_nc():
    global _CACHED_NC
    if _CACHED_NC is None:
        _CACHED_NC = _build_nc()
    return _CACHED_NC


def _prep_core_inputs(hidden_states, Wq, Wk, Wv, Wo):
    """Host-side shard + transpose + bf16 cast. Returns list of 8 input dicts."""
    xT_b = []
    for b in range(B):
        xT_b.append(np.ascontiguousarray(hidden_states[b].T).astype(BF16_NP))
    in_maps = []
    for c in range(N_CORES):
        b, g = divmod(c, TP)
        wq_rows = np.concatenate([
            Wq[(g * QH + h) * HEAD_DIM:(g * QH + h + 1) * HEAD_DIM, :]
            for h in HEAD_ORDER], axis=0)            # [512, H]
        wo_cols = np.concatenate([
            Wo[:, (g * QH + h) * HEAD_DIM:(g * QH + h + 1) * HEAD_DIM]
            for h in HEAD_ORDER], axis=1)            # [H, 512]
        in_maps.append({
            "xT": xT_b[b],
            "wqT": np.ascontiguousarray(wq_rows.T).astype(BF16_NP),
            "wkT": np.ascontiguousarray(Wk[g * KCH:(g + 1) * KCH, :].T).astype(BF16_NP),
            "wvT": np.ascontiguousarray(Wv[g * KCH:(g + 1) * KCH, :].T).astype(BF16_NP),
            "woT": np.ascontiguousarray(wo_cols.T).astype(BF16_NP),
        })
    return in_maps


def _combine(results):
    out = np.empty((B, S, H), dtype=np.float32)
    for b in range(B):
        acc = results[b * TP]["outT"].astype(np.float32)
        for g in range(1, TP):
            acc = acc + results[b * TP + g]["outT"]
        out[b] = acc.T
    return out


def kernel(hidden_states, attention_mask, Wq, Wk, Wv, Wo):
    # attention_mask is all zeros for this problem spec; softmax is invariant
    # to the zero additive mask, so it is not shipped to the device.
    hidden_states = np.asarray(hidden_states)
    nc = _get_nc()
    in_maps = _prep_core_inputs(hidden_states, np.asarray(Wq), np.asarray(Wk),
                                np.asarray(Wv), np.asarray(Wo))
    res = run_bass_kernel_spmd(nc, in_maps, list(range(N_CORES)))
    return _combine(res.results)


# revision 25
# speedup vs baseline: 1.1839x; 1.0039x over previous
"""Bitnet-style GQA attention block on 8 trn2 NeuronCores.

Sharding: DP2 (batch) x TP4 (heads). Each core handles one batch element and
8 q-heads / 2 kv-heads. Device layout is feature-major: activations live as
[channels, tokens]; all matmuls are bf16 with fp32 PSUM accumulation.

v3 design (vs the v1 baseline at ~428us):
- The scalar engine's exp stream (256 x [128,1024] ACTIVATEs ~= 285us busy) is
  the hard floor; everything is scheduled to keep it dense from ~16us on.
- Input DMA uses one 3D descriptor per block (multi-hk rearranged APs) so the
  queue-kick latency is negligible, ordered so K/Q-proj and the first scores
  start ~8-16us in. The vones ones-column memsets are emitted before any
  gpsimd-ring DMA so the first PV is not stuck behind descriptor kicks.
- Scores are row-tiled: per k-chunk, the two kv-heads run as two concurrent
  64-contraction matmuls on PE row halves (tile_position (0,0)/(64,0)),
  halving score PE time vs the zero-padded full-128 scheme.
- PV is operand-swapped: stationary = [V|1] token-major [128,65], moving =
  the exp tile [128,512] per head. Output accumulates attn.T[d,q] (+ a fused
  denominator row) directly in PSUM [65,512] - no per-chunk LDWEIGHTS of P
  tiles (v1 had 2048 of them) and no A-transposes before o-proj.
- PV for window w (a (qb,t) pair) runs one window late (lag-16) so V-proj
  can stream in behind the first windows; pa PSUM then needs only 2 banks.
  PSUM: scores ping-pong 2x[128,1024] (4) + pa 2x[65,512] (2) + scratch
  2x[128,512] (2) = 8 banks.
- Normalize: pa is freed by two quick DVE copies (attn rows + denom row);
  the rest (gpsimd partition_broadcast -> reciprocal_approx_fast -> multiply
  into the o-proj moving tiles) runs off the critical path. h=1's rows must
  land on SBUF partitions 64-127, which DVE cannot write (no partition
  shift), so h=1 bounces through a small SBUF->SBUF DMA.
- o-proj consumes attn.T tiles directly; output is bf16 (host sums partials
  in f32), halving output DMA.
"""

import numpy as np
import ml_dtypes
from contextlib import ExitStack

import concourse.bass as bass
import concourse.tile as tile
from concourse import bacc, mybir
from concourse.bass_utils import run_bass_kernel_spmd
from concourse.masks import make_identity

B, S, H = 2, 2048, 2048
N_HEADS, N_KV, HEAD_DIM = 32, 8, 64
N_CORES = 8
TP = 4                   # head-parallel degree per batch
QH = N_HEADS // TP       # 8 q-heads per core
KVH = N_KV // TP         # 2 kv heads per core
QCH = QH * HEAD_DIM      # 512
KCH = KVH * HEAD_DIM     # 128
ST = S // 128            # 16 k-token chunks
HK = H // 128            # 16 hidden-dim chunks
QB = 4                   # 512-wide q/token column blocks
HEAD_ORDER = [0, 4, 1, 5, 2, 6, 3, 7]  # slot j -> local q-head index

F32 = mybir.dt.float32
BF16 = mybir.dt.bfloat16
BF16_NP = ml_dtypes.bfloat16

_CACHED_NC = None


def _build_nc():
    nc = bacc.Bacc("TRN2", target_bir_lowering=False, debug=False,
                   num_devices=N_CORES)

    # host-packed layouts: per partition p, fully contiguous per-block runs
    # (>=2KB lines) so each block is one full-rate DMA descriptor.
    xTr = nc.dram_tensor("xTr", [128, 4 * HK * 512], BF16,
                         kind="ExternalInput").ap()   # [p][sb][hk][tok]
    wqr = nc.dram_tensor("wqr", [128, 4 * HK * 128], BF16,
                         kind="ExternalInput").ap()   # [p][t][hk][c]
    wkr = nc.dram_tensor("wkr", [128, HK * 128], BF16,
                         kind="ExternalInput").ap()   # [p][hk][c]
    wvr = nc.dram_tensor("wvr", [128, HK * 128], BF16,
                         kind="ExternalInput").ap()
    woT = nc.dram_tensor("woT", [QCH, H], BF16, kind="ExternalInput").ap()
    outT = nc.dram_tensor("outT", [H, S], BF16, kind="ExternalOutput").ap()

    with tile.TileContext(nc) as tc, ExitStack() as ctx:
        # ---- SBUF pools ----
        xp = ctx.enter_context(tc.tile_pool(name="xp", bufs=4))
        wqp = ctx.enter_context(tc.tile_pool(name="wqp", bufs=4))
        wkp = ctx.enter_context(tc.tile_pool(name="wkp", bufs=1))
        wvp = ctx.enter_context(tc.tile_pool(name="wvp", bufs=1))
        wop = ctx.enter_context(tc.tile_pool(name="wop", bufs=4))
        ktp = ctx.enter_context(tc.tile_pool(name="ktp", bufs=1))
        qtp = ctx.enter_context(tc.tile_pool(name="qtp", bufs=4))
        vp = ctx.enter_context(tc.tile_pool(name="vp", bufs=ST))
        pexp = ctx.enter_context(tc.tile_pool(name="pexp", bufs=20))
        atp = ctx.enter_context(tc.tile_pool(name="atp", bufs=8))
        stg = ctx.enter_context(tc.tile_pool(name="stg", bufs=4))
        nrm = ctx.enter_context(tc.tile_pool(name="nrm", bufs=2))
        cst = ctx.enter_context(tc.tile_pool(name="cst", bufs=1))
        # PSUM: scores ping-pong (4 banks) + pa (2) + scratch (2) = 8
        big = ctx.enter_context(tc.tile_pool(name="big", bufs=2, space="PSUM"))
        pap = ctx.enter_context(tc.tile_pool(name="pap", bufs=2, space="PSUM"))
        acc = ctx.enter_context(tc.tile_pool(name="acc", bufs=2, space="PSUM"))

        ident = cst.tile([128, 128], BF16, tag="ident")
        make_identity(nc, ident[:])
        ones64 = cst.tile([1, 64], BF16, tag="ones64")
        nc.vector.memset(ones64[:], 1.0)

        # prime the ACT exp table while input DMA streams
        dum = cst.tile([1, 2], F32, tag="dum")
        dumo = cst.tile([1, 2], BF16, tag="dumo")
        nc.vector.memset(dum[:], 1.0)
        nc.scalar.activation(dumo[:], dum[:],
                             mybir.ActivationFunctionType.Exp)

        # vones tiles; the ones columns are (re)written by DVE memsets AFTER
        # the V copies in emit_vproj - the PV stationary read's hazard
        # interval (reversed weights AP) reliably covers the ones columns,
        # and DVE is in-order, so that memset transitively orders the copies
        # before the PV LDWEIGHTS. (Without it, the hazard tracker misses
        # the copy ranges and the PV can read stale vones - seen on HW.)
        vones = [vp.tile([128, 130], BF16, tag="vones", name=f"vt{st}")
                 for st in range(ST)]

        # ---- input DMA: one 3D descriptor per block ----
        xts = []          # xts[sb]: [128, HK, 512]
        wqt = []          # wqt[t]: [128, HK, 128]
        wo = []

        wk_t = wkp.tile([128, HK, KCH], BF16, tag="wk")
        wv_t = wvp.tile([128, HK, KCH], BF16, tag="wv")
        for sb in range(4):
            t = xp.tile([128, HK, 512], BF16, tag="xt", name=f"xts{sb}")
            xts.append(t)
        for t_ in range(4):
            w = wqp.tile([128, HK, 128], BF16, tag="wq", name=f"wqt{t_}")
            wqt.append(w)

        # single sync ring, serial priority order (solo ring gets full HBM
        # rate; arrival order below matches first-use order)
        BLK = HK * 512
        QBLK = HK * 128
        nc.sync.dma_start(wk_t[:, :, :], wkr)
        nc.sync.dma_start(wqt[0][:, :, :], wqr[:, 0:QBLK])
        nc.sync.dma_start(xts[0][:, :, :], xTr[:, 0:BLK])
        nc.sync.dma_start(xts[1][:, :, :], xTr[:, BLK:2 * BLK])
        nc.sync.dma_start(wv_t[:, :, :], wvr)
        nc.sync.dma_start(wqt[1][:, :, :], wqr[:, QBLK:2 * QBLK])
        nc.sync.dma_start(xts[2][:, :, :], xTr[:, 2 * BLK:3 * BLK])
        nc.sync.dma_start(xts[3][:, :, :], xTr[:, 3 * BLK:4 * BLK])
        nc.sync.dma_start(wqt[2][:, :, :], wqr[:, 2 * QBLK:3 * QBLK])
        nc.sync.dma_start(wqt[3][:, :, :], wqr[:, 3 * QBLK:4 * QBLK])
        for i in range(4):
            t = wop.tile([128, H], BF16, tag="wo", name=f"wo{i}")
            nc.sync.dma_start(t[:], woT[i * 128:(i + 1) * 128, :])
            wo.append(t)

        # ---- persistent SBUF tensors ----
        kt_sb = ktp.tile([128, S], BF16, tag="kt")
        qt = [qtp.tile([128, S], BF16, tag="qt", name=f"qt{t_}")
              for t_ in range(4)]

        # ---- projection sub-blocks (emitted via the injection schedule) ----
        pk_h, pq_h, pvt_h, vtsb_h = {}, {}, {}, {}

        def guard(src_ap):
            # Tile elides LDWEIGHTS waits when an earlier PE-queue wait
            # covers the same semaphore count - unsound because the PE
            # hoists LDWEIGHTS past in-flight MATMULs (seen on HW: PV read
            # stale vones). This guard loads the freshly written bytes as
            # its *stationary* operand: the fresh dep can't be covered by
            # any earlier wait, so the guard's LDWEIGHTS carries it, and
            # later LDWEIGHTS can't hoist past another LDWEIGHTS.
            g = acc.tile([2, 2], F32, tag="acc", name="guard")
            nc.tensor.matmul(g[:], src_ap, ident[:, 0:2], start=True,
                             stop=True)

        def emit_kproj(sb, half):
            if half == 0:
                pk_h[sb] = acc.tile([128, 512], F32, tag="acc", name=f"pk{sb}")
            pk = pk_h[sb]
            for hk in range(half * 8, half * 8 + 8):
                nc.tensor.matmul(pk[:], wk_t[:, hk, :], xts[sb][:, hk, :],
                                 start=(hk == 0), stop=(hk == HK - 1),
                                 skip_group_check=True)
            if half == 1:
                nc.vector.tensor_copy(kt_sb[:, sb * 512:(sb + 1) * 512], pk[:])
                guard(kt_sb[:, sb * 512 + 510:sb * 512 + 512])

        def emit_qproj(t_, sb, half):
            if half == 0:
                pq_h[(t_, sb)] = acc.tile([128, 512], F32, tag="acc",
                                          name=f"pq{t_}_{sb}")
            pq = pq_h[(t_, sb)]
            for hk in range(half * 8, half * 8 + 8):
                nc.tensor.matmul(pq[:], wqt[t_][:, hk, :], xts[sb][:, hk, :],
                                 start=(hk == 0), stop=(hk == HK - 1),
                                 skip_group_check=True)
            if half == 1:
                nc.vector.tensor_copy(qt[t_][:, sb * 512:(sb + 1) * 512], pq[:])

        def emit_vproj(sb, part):
            if part == 0:
                pvt_h[sb] = acc.tile([128, 512], F32, tag="acc", name=f"pv{sb}")
            if part in (0, 1):
                pvt = pvt_h[sb]
                for hk in range(part * 8, part * 8 + 8):
                    nc.tensor.matmul(pvt[:], wv_t[:, hk, :], xts[sb][:, hk, :],
                                     start=(hk == 0), stop=(hk == HK - 1),
                                     skip_group_check=True)
            if part == 1:
                vtsb_h[sb] = stg.tile([128, 512], BF16, tag="vtsb",
                                      name=f"vtsb{sb}")
                nc.vector.tensor_copy(vtsb_h[sb][:], pvt_h[sb][:])
                guard(vtsb_h[sb][:, 510:512])
            if part == 2:
                vtsb = vtsb_h[sb]
                for j in range(4):
                    st = sb * 4 + j
                    ptr = acc.tile([128, 128], BF16, tag="acc", name="ptv")
                    nc.tensor.transpose(ptr[:], vtsb[:, j * 128:(j + 1) * 128],
                                        ident[:])
                    nc.vector.tensor_copy(vones[st][:, 0:64], ptr[:, 0:64])
                    nc.vector.tensor_copy(vones[st][:, 65:129], ptr[:, 64:128])
                    nc.vector.memset(vones[st][:, 64:65], 1.0)
                    nc.vector.memset(vones[st][:, 129:130], 1.0)
                guard(vones[sb * 4 + 3][:, 128:130])

        at_of = {}

        def emit_oproj_ot(qb, ot):
            po = acc.tile([128, 512], F32, tag="acc", name="po")
            for ak in range(4):
                nc.tensor.matmul(po[:], wo[ak][:, ot * 128:(ot + 1) * 128],
                                 at_of[qb][ak][:],
                                 start=(ak == 0), stop=(ak == 3),
                                 skip_group_check=True)
            so = stg.tile([128, 512], BF16, tag="so")
            nc.vector.tensor_copy(so[:], po[:])
            nc.sync.dma_start(
                outT[ot * 128:(ot + 1) * 128, qb * 512:(qb + 1) * 512], so[:])

        # ---- attention plumbing ----
        def emit_pv_group(ptiles, pa, kts):
            # attn.T[d,q] (+denominator row 64) accumulated over k chunks.
            # Grouped h-outer so consecutive matmuls accumulate into the SAME
            # psum bank - alternating banks per MM runs at isolated-MM
            # latency (~405ns) instead of the pipelined rate (~215ns).
            for h in range(2):
                for kt in kts:
                    nc.tensor.matmul(pa[h][:, :],
                                     vones[kt][:, h * 65:h * 65 + 65],
                                     ptiles[kt][:, h * 512:(h + 1) * 512],
                                     start=(kt == 0), stop=(kt == ST - 1),
                                     skip_group_check=True)

        def emit_normalize(pqb, pt_, pa):
            # Free pa quickly with two copies; the recip+multiply runs off
            # the critical path. at[qb][t] rows h*64.. get pa[h] rows 0:64.
            for h in range(2):
                anT = nrm.tile([64, 512], F32, tag="anT")
                nc.vector.tensor_copy(anT[:], pa[h][0:64, :])
                dcp = nrm.tile([1, 512], BF16, tag="dcp")
                nc.vector.tensor_copy(dcp[:], pa[h][64:65, :])
                # replicate the denominator row across 64 partitions via a
                # K=1 matmul (ones outer product) - cheap, fully tracked
                pbc = acc.tile([64, 512], F32, tag="acc", name="pbc")
                nc.tensor.matmul(pbc[:], ones64[:], dcp[:],
                                 start=True, stop=True)
                rct = nrm.tile([64, 512], F32, tag="rct")
                nc.vector.reciprocal(rct[:], pbc[:])
                if h == 0:
                    nc.vector.tensor_mul(at_of[pqb][pt_][0:64, :],
                                         anT[:], rct[:])
                else:
                    # DVE cannot shift partitions; bounce via SBUF DMA
                    tmp = nrm.tile([64, 512], BF16, tag="tmp")
                    nc.vector.tensor_mul(tmp[:], anT[:], rct[:])
                    nc.sync.dma_start(at_of[pqb][pt_][64:128, :], tmp[:])

        # ---- injection schedule: (qb,t) -> {chunk: [thunks]} ----
        def KB(sb, h):
            return lambda: emit_kproj(sb, h)

        def QP(t_, sb, h):
            return lambda: emit_qproj(t_, sb, h)

        def VB(sb, p):
            return lambda: emit_vproj(sb, p)

        def OB(qb, ot):
            return lambda: emit_oproj_ot(qb, ot)

        def qw(t_, sb):     # standard Q-proj pair placement
            return {1: [QP(t_, sb, 0)], 2: [QP(t_, sb, 1)]}

        def ow(qb, lo, hi, base=4):  # o-proj row-tiles spread over chunks
            out = {}
            for i, ot in enumerate(range(lo, hi)):
                out.setdefault(min(base + 2 * i, ST - 2), []).append(OB(qb, ot))
            return out

        def merge(*ds):
            out = {}
            for dd in ds:
                for k, v in dd.items():
                    out.setdefault(k, []).extend(v)
            return out

        sched = {
            (0, 0): {1: [KB(1, 0)], 2: [KB(1, 1)], 5: [KB(2, 0)],
                     6: [KB(2, 1)], 9: [KB(3, 0)], 10: [KB(3, 1)],
                     12: [QP(1, 0, 0)], 13: [QP(1, 0, 1)],
                     14: [VB(0, 0)], 15: [VB(0, 1)]},
            # PV(prev, kt) is emitted at chunk kt BEFORE the chunk's thunks:
            # VB(sb, 2) (writes vones[4sb..4sb+3]) must sit at chunk < 4*sb.
            (0, 1): {0: [VB(1, 0)], 1: [VB(1, 1)], 2: [VB(1, 2)],
                     3: [QP(2, 0, 0)], 4: [VB(2, 0)], 5: [VB(2, 1)],
                     6: [VB(2, 2)], 7: [QP(2, 0, 1)],
                     8: [VB(3, 0)], 9: [VB(3, 1)], 10: [VB(3, 2)]},
            (0, 2): qw(3, 0),
            (0, 3): qw(0, 1),
            (1, 0): qw(1, 1),
            (1, 1): merge(qw(2, 1), ow(0, 0, 5)),
            (1, 2): merge(qw(3, 1), ow(0, 5, 10)),
            (1, 3): merge(qw(0, 2), ow(0, 10, 16), ),
            (2, 0): qw(1, 2),
            (2, 1): merge(qw(2, 2), ow(1, 0, 5)),
            (2, 2): merge(qw(3, 2), ow(1, 5, 10)),
            (2, 3): merge(qw(0, 3), ow(1, 10, 16)),
            (3, 0): qw(1, 3),
            (3, 1): merge(qw(2, 3), ow(2, 0, 5)),
            (3, 2): merge(qw(3, 3), ow(2, 5, 10)),
            (3, 3): ow(2, 10, 16, base=2),
        }
        # (0,0) tail: V0 part 2 runs right after the chunk loop
        late_of = {(0, 0): [VB(0, 2)]}

        # ---- pre-loop: K sb0 and Q(0,0) ----
        emit_kproj(0, 0)
        emit_kproj(0, 1)
        emit_qproj(0, 0, 0)
        emit_qproj(0, 0, 1)

        # ---- main loop: qb outer, head-pair t inner; PV runs one window late
        prev = None  # (ptiles, qb, t)
        for qb in range(QB):
            at_of[qb] = [atp.tile([128, 512], BF16, tag="at",
                                  name=f"at{qb}_{ak}") for ak in range(4)]
            for t_ in range(4):
                cmap = sched.get((qb, t_), {})
                cur = []
                if prev is not None:
                    pa = [pap.tile([65, 512], F32, tag="pa", name=f"pa{h}")
                          for h in range(2)]
                for kt in range(ST):
                    ps2 = big.tile([128, 1024], F32, tag="big")
                    for h in range(2):
                        nc.tensor.matmul(
                            ps2[:, h * 512:(h + 1) * 512],
                            kt_sb[h * 64:(h + 1) * 64,
                                  kt * 128:(kt + 1) * 128],
                            qt[t_][h * 64:(h + 1) * 64,
                                   qb * 512:(qb + 1) * 512],
                            start=True, stop=True)
                    pe = pexp.tile([128, 1024], BF16, tag="pexp")
                    nc.scalar.activation(pe[:], ps2[:],
                                         mybir.ActivationFunctionType.Exp,
                                         scale=0.125)
                    cur.append(pe)
                    if prev is not None and kt % 4 == 3:
                        emit_pv_group(prev[0], pa, range(kt - 3, kt + 1))
                    for f in cmap.get(kt, []):
                        f()
                for f in late_of.get((qb, t_), []):
                    f()
                if prev is not None:
                    emit_normalize(prev[1], prev[2], pa)
                prev = (cur, qb, t_)

        # ---- tail: PV + normalize of the last window, then o-proj qb3 ----
        pa = [pap.tile([65, 512], F32, tag="pa", name=f"paz{h}")
              for h in range(2)]
        emit_pv_group(prev[0], pa, range(ST))
        emit_normalize(prev[1], prev[2], pa)
        for ot in range(HK):
            emit_oproj_ot(QB - 1, ot)

    nc.compile()
    return nc


def _get_nc():
    global _CACHED_NC
    if _CACHED_NC is None:
        _CACHED_NC = _build_nc()
    return _CACHED_NC


def _pack_p_major(mT, ncols_groups):
    """[H, G*C] channel-major -> [128, G*HK*C] per-partition contiguous
    [p][g][hk][c] layout (full-rate DMA lines)."""
    Hdim, W = mT.shape
    C = W // ncols_groups
    a = mT.reshape(HK, 128, ncols_groups, C)         # [hk, p, g, c]
    a = a.transpose(1, 2, 0, 3)                      # [p, g, hk, c]
    return np.ascontiguousarray(a.reshape(128, ncols_groups * HK * C))


def _prep_core_inputs(hidden_states, Wq, Wk, Wv, Wo):
    """Host-side shard + pack + bf16 cast. Returns list of 8 input dicts."""
    xr_b = []
    for b in range(B):
        xT = hidden_states[b].T.astype(BF16_NP)      # [H, S]
        xr_b.append(_pack_p_major(xT, 4))            # [p][sb][hk][512]
    in_maps = []
    for c in range(N_CORES):
        b, g = divmod(c, TP)
        wq_rows = np.concatenate([
            Wq[(g * QH + h) * HEAD_DIM:(g * QH + h + 1) * HEAD_DIM, :]
            for h in HEAD_ORDER], axis=0)            # [512, H]
        wo_cols = np.concatenate([
            Wo[:, (g * QH + h) * HEAD_DIM:(g * QH + h + 1) * HEAD_DIM]
            for h in HEAD_ORDER], axis=1)            # [H, 512]
        in_maps.append({
            "xTr": xr_b[b],
            "wqr": _pack_p_major(wq_rows.T.astype(BF16_NP), 4),
            "wkr": _pack_p_major(Wk[g * KCH:(g + 1) * KCH, :].T.astype(BF16_NP), 1),
            "wvr": _pack_p_major(Wv[g * KCH:(g + 1) * KCH, :].T.astype(BF16_NP), 1),
            "woT": np.ascontiguousarray(wo_cols.T).astype(BF16_NP),
        })
    return in_maps


def _combine(results):
    out = np.empty((B, S, H), dtype=np.float32)
    for b in range(B):
        acc = results[b * TP]["outT"].astype(np.float32)
        for g in range(1, TP):
            acc = acc + results[b * TP + g]["outT"]
        out[b] = acc.T
    return out


def kernel(hidden_states, attention_mask, Wq, Wk, Wv, Wo):
    # attention_mask is all zeros for this problem spec; softmax is invariant
    # to the zero additive mask, so it is not shipped to the device.
    hidden_states = np.asarray(hidden_states)
    nc = _get_nc()
    in_maps = _prep_core_inputs(hidden_states, np.asarray(Wq), np.asarray(Wk),
                                np.asarray(Wv), np.asarray(Wo))
    res = run_bass_kernel_spmd(nc, in_maps, list(range(N_CORES)))
    return _combine(res.results)
